# revision 1
# baseline (speedup 1.0000x reference)
"""Trainium2 Bass kernel for nn_MoE_AllToAll_Layer (top-1 MoE, 8 experts).

Expert parallel across 8 NeuronCores: core e holds expert e's FFN weights.
Each core (replicated) computes the router + stable counting sort on device,
scatters (token_id, score) records into a sorted-position-indexed DRAM array
with ONE dma_scatter_add, gathers its own expert's rows with dma_gather,
runs the expert FFN on the compacted tokens, and writes compact scaled
output rows + token ids. The host places rows back by token id (pure data
movement).

Validated-on-HW design notes:
 - The router is exact fp32 (an fp16 router flips ~1 argmax on these inputs,
   and one flip shifts the reference's positional score permutation, which
   corrupts hundreds of rows). x is streamed fp32 but used as the PE's
   stationary operand with the tiny Wr moving, so fp32's 4 cycles/row apply
   only to 8-wide outputs: the whole router is a few us of PE time.
 - Softmax/argmax/sort-mask work runs per 512-token block, overlapped with
   the x stream; the counting sort's expert offsets are seeded into the
   Hillis-Steele scan so the final position computation is three wide vector
   ops instead of a per-expert loop.
 - FFN weights/activations/outputs are fp16 (1 cycle/row, half the DMA).
 - dma_gather/dma_scatter_add index layout: [128, n/16] int16, the
   [16, n/16] wrap (slot i at [i%16, i//16]) replicated 8x down partitions
   (one copy per Q7 core); built on the PE with 8 selection matmuls.
   Gather writes slot i to out[i%128, i//128].
 - The reference's positional score scale is folded into the PE transpose
   of the gathered x rows via a plain matmul with diag(score) as the rhs.
 - FFN: layer-1 for all hidden blocks (streaming W1), then layer-2
   accumulating all 32 hidden chunks in PSUM (W2 prefetched during L1).
   Both layers keep the stationary weight tile across the two token chunks
   to halve Ldweights issue cost.
"""

import numpy as np
import sys

sys.path.insert(0, "/opt/trn_rl_repo")

import concourse.bass as bass  # noqa: E402
import concourse.tile as tile  # noqa: E402
from concourse import bacc, mybir  # noqa: E402
from concourse.bass_utils import run_bass_kernel_spmd  # noqa: E402

P = 128
N_TOKENS = 4096
D_IN = 1024
D_HID = 4096
D_OUT = 1024
E = 8
NT = N_TOKENS // P          # 32 token tiles
DC = D_IN // P              # 8 d-chunks
KC = D_OUT // P             # 8 k-chunks
JG = D_HID // P             # 32 hidden chunks
CAP = 640                   # per-expert token capacity (max actual count 537)
RT = CAP // P               # 5 row tiles
NB = 8                      # router token blocks of 512
BT = N_TOKENS // NB         # 512 tokens per block
TPB = BT // P               # 4 token tiles per block
NJB = 8                     # W1 streaming blocks (512 hidden each)
JB = D_HID // NJB           # 512
JCB = JB // P               # 4 hidden chunks per W1 block
NWB = 4                     # W2 streaming blocks (8 hidden chunks each)
CHUNKS = ((0, 512), (512, CAP - 512))   # token chunks for FFN matmuls
REC = 64                    # f32 row stride of sidx records (256B min)

dt = mybir.dt
Alu = mybir.AluOpType
Act = mybir.ActivationFunctionType
Ax = mybir.AxisListType

f32 = dt.float32
f16 = dt.float16
i16 = dt.int16

# const blob column offsets
C16_IDENT = 0            # [0:128)   eye(128) fp16
C16_TRI = 128            # [128:256) tri[q,p] = q < p
C16_ONES = 256           # [256:257) 1.0
C16_W = 257
CF_IOTAC = 0             # [0:32)  iotac[p, t] = t*128 + p
CF_IOTAW = 32            # [32:72) iotaw[p, m] = 16*m + p%16 (wrapped iota)
CF_WR = 72               # [72:136) wr32[p, c*8+e] = Wr[c*128+p, e] (fp32!)
CF_W = 136
CR_ONES = 0              # [0:128) ones
CR_ONEHOT = 128          # [128:136) onehot(core expert)
CR_W = 136


def build_nc():
    nc = bacc.Bacc(
        "TRN2",
        target_bir_lowering=False,
        debug=False,
        enable_asserts=False,
        num_devices=E,
    )

    x32t = nc.dram_tensor("x32t", [P, NB, DC, BT], f32, kind="ExternalInput").ap()
    x16 = nc.dram_tensor("x16", [N_TOKENS, D_IN], f16, kind="ExternalInput").ap()
    w1t = nc.dram_tensor("w1t", [P, NJB, DC, JB], f16, kind="ExternalInput").ap()
    w2t = nc.dram_tensor("w2t", [P, NWB, JG // NWB, D_OUT], f16, kind="ExternalInput").ap()
    c16 = nc.dram_tensor("c16", [P, C16_W], f16, kind="ExternalInput").ap()
    cf32 = nc.dram_tensor("cf32", [P, CF_W], f32, kind="ExternalInput").ap()
    cr32 = nc.dram_tensor("cr32", [1, CR_W], f32, kind="ExternalInput").ap()
    # pre-zeroed scatter destination (host ships zeros)
    sidx = nc.dram_tensor("sidx", [N_TOKENS, REC], f32, kind="ExternalInput").ap()
    # wrap/replicate selector: selg[p, g, q] = 1 iff p == 16*g + (q % 16)
    selg = nc.dram_tensor("selg", [P, 8, P], f32, kind="ExternalInput").ap()

    outT16 = nc.dram_tensor("outT16", [P, KC, CAP], f16, kind="ExternalOutput").ap()
    ids5 = nc.dram_tensor("ids5", [P, RT], f32, kind="ExternalOutput").ap()
    cnts = nc.dram_tensor("cnts", [1, E], f32, kind="ExternalOutput").ap()

    with tile.TileContext(nc) as tc:
        emit(nc, tc, locals())
    nc.compile()
    return nc


def emit(nc, tc, io):
    x32t, x16, w1t, w2t = io["x32t"], io["x16"], io["w1t"], io["w2t"]
    c16, cf32, cr32 = io["c16"], io["cf32"], io["cr32"]
    outT16, ids5, cnts = io["outT16"], io["ids5"], io["cnts"]
    sidx = io["sidx"]

    with tc.tile_pool(name="consts", bufs=1) as cpool:
        c16_sb = cpool.tile([P, C16_W], f16, tag="c16")
        nc.sync.dma_start(c16_sb[:], c16)
        cf_sb = cpool.tile([P, CF_W], f32, tag="cf32")
        nc.sync.dma_start(cf_sb[:], cf32)
        cr_sb = cpool.tile([1, CR_W], f32, tag="cr32")
        nc.sync.dma_start(cr_sb[:], cr32)
        selg_sb = cpool.tile([P, 8, P], f32, tag="selg")

        ident16 = c16_sb[:, C16_IDENT:C16_IDENT + P]
        tri16 = c16_sb[:, C16_TRI:C16_TRI + P]
        ones1_16 = c16_sb[:, C16_ONES:C16_ONES + 1]
        iotac = cf_sb[:, CF_IOTAC:CF_IOTAC + NT]
        iotaw = cf_sb[:, CF_IOTAW:CF_IOTAW + RT * 8]
        wr32 = cf_sb[:, CF_WR:CF_WR + DC * E]
        onesr = cr_sb[:, CR_ONES:CR_ONES + P]
        onehot = cr_sb[:, CR_ONEHOT:CR_ONEHOT + E]

        with tc.tile_pool(name="persist", bufs=1) as pp:
            lg_all = pp.tile([P, NT, E], f32, tag="lgall")
            mx = pp.tile([P, NT], f32, tag="mx")
            score = pp.tile([P, NT], f32, tag="score")
            renc = pp.tile([P, NT], f32, tag="renc")
            m_all = pp.tile([P, NT, E], f32, tag="mall")
            m16 = pp.tile([P, NT * E], f16, tag="m16")
            own_bc = pp.tile([P, 1], f32, tag="ownbc")
            xT_all = pp.tile([P, DC, CAP], f16, tag="xTall")
            hT_all = pp.tile([P, JG, CAP], f16, tag="hTall")
            o16 = pp.tile([P, KC, CAP], f16, tag="o16")
            s2ro = pp.tile([P, RT, REC], f32, tag="s2ro")
            w2_all = pp.tile([P, JG, D_OUT], f16, tag="w2all")
            sc_big = pp.tile([P, NT, 2], f32, tag="scbig")

            # scatter records: col0 = token id (known now), col1 = score
            nc.vector.tensor_copy(out=sc_big[:, :, 0], in_=iotac)

            # ---------------- router: exact fp32, x stationary ---------------
            # logits[tok, e] with x chunks as the (free) PE weights and the
            # tiny Wr as the moving operand: fp32's 4 cyc/row applies only to
            # the 8-wide output rows. Softmax/argmax/sort-mask per block,
            # overlapped with the stream.
            with (
                tc.tile_pool(name="rwork", bufs=2) as rp,
                tc.tile_pool(name="tpsum", bufs=3, space="PSUM") as tps,
            ):
                for b in range(NB):
                    xtt = rp.tile([P, DC, BT], f32, tag="xtt")
                    nc.sync.dma_start(xtt[:], x32t[:, b])
                    for i in range(TPB):
                        t = b * TPB + i
                        lg_ps = tps.tile([P, 8], f32, tag="lgps")
                        for c in range(DC):
                            nc.tensor.matmul(
                                lg_ps[:],
                                lhsT=xtt[:, c, i * P:(i + 1) * P],
                                rhs=wr32[:].rearrange(
                                    "p (c e) -> p c e", c=DC)[:, c, :],
                                start=(c == 0), stop=(c == DC - 1),
                            )
                        nc.vector.tensor_copy(out=lg_all[:, t, :], in_=lg_ps[:])
                    sl = slice(b * TPB, (b + 1) * TPB)
                    nc.vector.tensor_reduce(
                        out=mx[:, sl], in_=lg_all[:, sl, :], axis=Ax.X,
                        op=Alu.max,
                    )
                    # renc = max_e (lg == mx) * (E - e)  (first-max tiebreak)
                    for e in range(E):
                        eq = rp.tile([P, TPB], f32, tag="eq", bufs=2)
                        nc.vector.tensor_tensor(
                            out=eq[:], in0=lg_all[:, sl, e], in1=mx[:, sl],
                            op=Alu.is_equal,
                        )
                        if e == 0:
                            nc.vector.tensor_scalar_mul(
                                renc[:, sl], eq[:], float(E)
                            )
                        else:
                            eqr = rp.tile([P, TPB], f32, tag="eqr", bufs=2)
                            nc.vector.tensor_scalar_mul(
                                eqr[:], eq[:], float(E - e)
                            )
                            nc.vector.tensor_tensor(
                                out=renc[:, sl], in0=renc[:, sl], in1=eqr[:],
                                op=Alu.max,
                            )
                    el = rp.tile([P, TPB, E], f32, tag="el", bufs=2)
                    nc.scalar.activation(el[:], lg_all[:, sl, :], Act.Exp)
                    ssum = rp.tile([P, TPB], f32, tag="ssum", bufs=2)
                    nc.vector.tensor_reduce(
                        out=ssum[:], in_=el[:], axis=Ax.X, op=Alu.add
                    )
                    emx = rp.tile([P, TPB], f32, tag="emx", bufs=2)
                    nc.scalar.activation(emx[:], mx[:, sl], Act.Exp)
                    rsum = rp.tile([P, TPB], f32, tag="rsum", bufs=2)
                    nc.vector.reciprocal(rsum[:], ssum[:])
                    nc.vector.tensor_tensor(
                        out=score[:, sl], in0=emx[:], in1=rsum[:], op=Alu.mult
                    )
                    nc.vector.tensor_copy(
                        out=sc_big[:, sl, 1], in_=score[:, sl]
                    )
                    # sort mask, t-major
                    for e in range(E):
                        nc.vector.tensor_scalar(
                            out=m_all[:, sl, e], in0=renc[:, sl],
                            scalar1=float(E - e), scalar2=None,
                            op0=Alu.is_equal,
                        )
                    nc.vector.tensor_copy(
                        out=m16[:, b * TPB * E:(b + 1) * TPB * E],
                        in_=m_all[:, sl, :],
                    )

            # selector consts load late: keeps the early DMA queue free for
            # the router x stream; only needed once dest is ready
            nc.sync.dma_start(selg_sb[:], io["selg"])

            # ---------------- stable counting sort ----------------
            with (
                tc.tile_pool(name="swork", bufs=1) as sw,
                tc.tile_pool(name="spsum", bufs=1, space="PSUM") as sps,
            ):
                prefix_ps = sps.tile([P, NT, E], f32, tag="prefix")
                nc.tensor.matmul(
                    prefix_ps[:].rearrange("p t e -> p (t e)"),
                    lhsT=tri16, rhs=m16[:], start=True, stop=True,
                )
                colsum_ps = sps.tile([1, NT * E], f32, tag="colsum")
                nc.tensor.matmul(
                    colsum_ps[:], lhsT=ones1_16, rhs=m16[:],
                    start=True, stop=True,
                )
                cs = sw.tile([1, NT, E], f32, tag="cs")
                nc.vector.tensor_copy(
                    out=cs[:].rearrange("p t e -> p (t e)"), in_=colsum_ps[:]
                )
                # counts independent of the scan: reduce over t
                csT = sw.tile([1, E, NT], f32, tag="csT")
                nc.vector.tensor_copy(
                    out=csT[:], in_=cs[:].rearrange("p t e -> p e t")
                )
                cnt_row = sw.tile([1, E], f32, tag="cnt")
                nc.vector.tensor_reduce(
                    out=cnt_row[:], in_=csT[:], axis=Ax.X, op=Alu.add
                )
                nc.scalar.dma_start(cnts, cnt_row[:])

                # exclusive prefix over experts -> global offsets
                ocur = sw.tile([1, E], f32, tag="off0")
                nc.vector.memset(ocur[:], 0.0)
                nc.vector.tensor_copy(out=ocur[:, 1:E], in_=cnt_row[:, 0:E - 1])
                for i, s in enumerate([1, 2, 4]):
                    onxt = sw.tile([1, E], f32, tag=f"off{i + 1}")
                    nc.vector.tensor_tensor(
                        out=onxt[:, s:E], in0=ocur[:, s:E],
                        in1=ocur[:, 0:E - s], op=Alu.add,
                    )
                    nc.vector.tensor_copy(out=onxt[:, 0:s], in_=ocur[:, 0:s])
                    ocur = onxt
                off_row = ocur  # [1, E]

                oh = sw.tile([1, E], f32, tag="oh")
                nc.vector.tensor_tensor(
                    out=oh[:], in0=off_row[:], in1=onehot, op=Alu.mult
                )
                own1 = sw.tile([1, 1], f32, tag="own1")
                nc.vector.tensor_reduce(
                    out=own1[:], in_=oh[:], axis=Ax.X, op=Alu.add
                )
                ownb_ps = sps.tile([P, 1], f32, tag="ownb")
                nc.tensor.matmul(
                    ownb_ps[:], lhsT=onesr, rhs=own1[:], start=True, stop=True
                )
                nc.vector.tensor_copy(out=own_bc[:], in_=ownb_ps[:])

                # within-expert exclusive prefix over t, SEEDED with the
                # global expert offsets so carry2 = off_e + sum_{t'<t} cs
                cur = sw.tile([1, NT, E], f32, tag="hs0")
                nc.vector.tensor_copy(out=cur[:, 0:1, :], in_=off_row[:])
                nc.vector.tensor_copy(
                    out=cur[:, 1:NT, :], in_=cs[:, 0:NT - 1, :]
                )
                for i, s in enumerate([1, 2, 4, 8, 16]):
                    nxt = sw.tile([1, NT, E], f32, tag=f"hs{i + 1}")
                    nc.vector.tensor_tensor(
                        out=nxt[:, s:NT, :], in0=cur[:, s:NT, :],
                        in1=cur[:, 0:NT - s, :], op=Alu.add,
                    )
                    nc.vector.tensor_copy(out=nxt[:, 0:s, :], in_=cur[:, 0:s, :])
                    cur = nxt
                carry2 = cur  # [1, t, e] = off_e + exclusive within-e prefix

                carb_ps = sps.tile([P, NT, E], f32, tag="carb")
                nc.tensor.matmul(
                    carb_ps[:].rearrange("p t e -> p (t e)"),
                    lhsT=onesr, rhs=carry2[:].rearrange("p t e -> p (t e)"),
                    start=True, stop=True,
                )

                # dest[p, t] = sum_e m_e * (prefix_e + carry2_e)
                # (hardware allows only one PSUM input per vector op)
                carb_sb = sw.tile([P, NT, E], f32, tag="carbsb")
                nc.vector.tensor_copy(out=carb_sb[:], in_=carb_ps[:])
                s1 = sw.tile([P, NT, E], f32, tag="s1")
                nc.vector.tensor_tensor(
                    out=s1[:], in0=prefix_ps[:], in1=carb_sb[:], op=Alu.add
                )
                s2 = sw.tile([P, NT, E], f32, tag="s2")
                nc.vector.tensor_tensor(
                    out=s2[:], in0=s1[:], in1=m_all[:], op=Alu.mult
                )
                dest = sw.tile([P, NT], f32, tag="dest")
                nc.vector.tensor_reduce(
                    out=dest[:], in_=s2[:], axis=Ax.X, op=Alu.add
                )

                # wrap + replicate scatter indices on the PE: slot i = t*128+p
                # lives at [i%16 (+16c), t*8 + p//16]; the selection matmul
                # moves dest[16g + q%16, t] to partition q, column group g,
                # replicated for all 8 Q7 cores at once.
                dest16w = sw.tile([P, NT, 8], i16, tag="dest16w")
                with tc.tile_pool(name="wps", bufs=2, space="PSUM") as wps:
                    for g in range(8):
                        wp_ps = wps.tile([P, NT], f32, tag="wpps")
                        nc.tensor.matmul(
                            wp_ps[:], lhsT=selg_sb[:, g, :], rhs=dest[:],
                            start=True, stop=True,
                        )
                        nc.vector.tensor_copy(
                            out=dest16w[:, :, g], in_=wp_ps[:]
                        )
                nc.gpsimd.dma_scatter_add(
                    sidx[:, 0:2], sc_big[:],
                    dest16w[:].rearrange("p t g -> p (t g)"),
                    N_TOKENS, N_TOKENS, 2, elem_step=REC,
                )

            # ---------------- gather own rows + scaled transpose -------------
            with tc.tile_pool(name="gwork", bufs=1) as gp:
                # own sorted positions, wrapped layout, computed in place
                posw = gp.tile([P, RT * 8], f32, tag="posw")
                nc.vector.tensor_scalar(
                    out=posw[:], in0=iotaw,
                    scalar1=own_bc[:, 0:1], scalar2=float(N_TOKENS - 1),
                    op0=Alu.add, op1=Alu.min,
                )
                pos16w = gp.tile([P, RT * 8], i16, tag="pos16w")
                nc.vector.tensor_copy(out=pos16w[:], in_=posw[:])
                sgo = gp.tile([P, RT, REC], f32, tag="sgo")
                nc.gpsimd.dma_gather(
                    sgo[:], sidx, pos16w[:], CAP, CAP, REC,
                )
                nc.scalar.dma_start(ids5, sgo[:, :, 0])
                ids16w = gp.tile([P, RT, 8], i16, tag="ids16w")
                with tc.tile_pool(name="iwps", bufs=2, space="PSUM") as iwps:
                    for g in range(8):
                        iw_ps = iwps.tile([P, RT], f32, tag="iwps")
                        nc.tensor.matmul(
                            iw_ps[:], lhsT=selg_sb[:, g, :], rhs=sgo[:, :, 0],
                            start=True, stop=True,
                        )
                        nc.vector.tensor_copy(
                            out=ids16w[:, :, g], in_=iw_ps[:]
                        )
                # scale lookup: sorted_scores[token_id]
                nc.gpsimd.dma_gather(
                    s2ro[:], sidx,
                    ids16w[:].rearrange("p r g -> p (r g)"), CAP, CAP, REC,
                )
                xg = gp.tile([P, RT, D_IN], f16, tag="xg")
                idsw_flat = ids16w[:].rearrange("p r g -> p (r g)")
                nc.gpsimd.dma_gather(
                    xg[:, 0:3, :], x16, idsw_flat[:, 0:24], 384, 384, D_IN,
                )
                nc.gpsimd.dma_gather(
                    xg[:, 3:RT, :], x16, idsw_flat[:, 24:40], 256, 256, D_IN,
                )
                # gate the W2 prefetch behind xg: without this the scheduler
                # hoists 23us of W2 transfers ahead of the scatter/gather
                # chain and the W1 stream, stalling both
                nc.vector.tensor_copy(
                    out=w2_all[0:1, 0:1, 0:1], in_=xg[0:1, 0:1, 0:1]
                )
                # transpose gathered rows, folding the positional score scale
                # in by multiplying with diag(score) on the PE
                # (exact because b1 = b2 = 0 and scores > 0)
                with tc.tile_pool(name="tpx", bufs=8, space="PSUM") as tpx:
                    for rt in range(RT):
                        diag = gp.tile([P, P], f16, tag="diag", bufs=2)
                        nc.vector.tensor_scalar(
                            out=diag[:], in0=ident16,
                            scalar1=s2ro[:, rt, 1:2], scalar2=None, op0=Alu.mult,
                        )
                        for c in range(DC):
                            tp = tpx.tile([P, P], f32, tag="tp")
                            nc.tensor.matmul(
                                tp[:],
                                lhsT=xg[:, rt, c * P:(c + 1) * P],
                                rhs=diag[:],
                                start=True, stop=True,
                            )
                            if c % 2 == 0:
                                nc.vector.tensor_copy(
                                    out=xT_all[:, c, rt * P:(rt + 1) * P],
                                    in_=tp[:],
                                )
                            else:
                                nc.scalar.activation(
                                    xT_all[:, c, rt * P:(rt + 1) * P],
                                    tp[:], Act.Copy,
                                )

            # ---------------- FFN layer 1 (stream W1, W2 prefetch) -----------
            # c-outer so both token chunks reuse the stationary W1 tile
            with (
                tc.tile_pool(name="w1pool", bufs=3) as wp,
                tc.tile_pool(name="l1ps", bufs=3, space="PSUM") as l1ps,
                tc.tile_pool(name="l1tail", bufs=3, space="PSUM") as l1tail,
            ):
                for jb in range(NJB):
                    w1b = wp.tile([P, DC, JB], f16, tag="w1b")
                    nc.sync.dma_start(w1b[:], w1t[:, jb])
                    if 3 <= jb <= 6:
                        wb = jb - 3
                        nc.sync.dma_start(
                            w2_all[:, wb * 8:(wb + 1) * 8, :], w2t[:, wb]
                        )
                    for jc in range(JCB):
                        jg = jb * JCB + jc
                        ps_a = l1ps.tile([P, 512], f32, tag="l1pa")
                        ps_b = l1tail.tile([P, CAP - 512], f32, tag="l1pb")
                        for c in range(DC):
                            lhsT = w1b[:, c, jc * P:(jc + 1) * P]
                            nc.tensor.matmul(
                                ps_a[:], lhsT=lhsT,
                                rhs=xT_all[:, c, 0:512],
                                start=(c == 0), stop=(c == DC - 1),
                            )
                            nc.tensor.matmul(
                                ps_b[:], lhsT=lhsT,
                                rhs=xT_all[:, c, 512:CAP],
                                start=(c == 0), stop=(c == DC - 1),
                            )
                        nc.scalar.activation(
                            hT_all[:, jg, 0:512], ps_a[:], Act.Relu
                        )
                        nc.scalar.activation(
                            hT_all[:, jg, 512:CAP], ps_b[:], Act.Relu
                        )

            # ---------------- FFN layer 2 (full PSUM accumulation) -----------
            # g-outer with both chunks inner: one Ldweights per (g, kc)
            with (
                tc.tile_pool(name="l2ps", bufs=2, space="PSUM") as l2ps,
                tc.tile_pool(name="l2tail", bufs=2, space="PSUM") as l2tail,
            ):
                for kc in range(KC):
                    ps_a = l2ps.tile([P, 512], f32, tag="l2pa")
                    ps_b = l2tail.tile([P, CAP - 512], f32, tag="l2pb")
                    for g in range(JG):
                        lhsT = w2_all[:, g, kc * P:(kc + 1) * P]
                        nc.tensor.matmul(
                            ps_a[:], lhsT=lhsT, rhs=hT_all[:, g, 0:512],
                            start=(g == 0), stop=(g == JG - 1),
                        )
                        nc.tensor.matmul(
                            ps_b[:], lhsT=lhsT, rhs=hT_all[:, g, 512:CAP],
                            start=(g == 0), stop=(g == JG - 1),
                        )
                    nc.vector.tensor_copy(out=o16[:, kc, 0:512], in_=ps_a[:])
                    nc.vector.tensor_copy(out=o16[:, kc, 512:CAP], in_=ps_b[:])
                    nc.sync.dma_start(outT16[:, kc, :], o16[:, kc, :])


_NC_CACHE = None


def _get_nc():
    global _NC_CACHE
    if _NC_CACHE is None:
        _NC_CACHE = build_nc()
    return _NC_CACHE


def _make_in_maps(x, Wr, br, W1, b1, W2, b2):
    x = np.asarray(x, np.float32)
    Wr = np.asarray(Wr, np.float32)
    br = np.asarray(br, np.float32)
    W1 = np.asarray(W1, np.float32)
    W2 = np.asarray(W2, np.float32)
    b1 = np.asarray(b1, np.float32)
    b2 = np.asarray(b2, np.float32)
    # the kernel folds the positional score scale onto x and drops the FFN
    # bias adds, which is exact only for zero biases (the spec generates
    # zeros)
    assert not np.any(b1) and not np.any(b2), "nonzero FFN biases unsupported"
    assert not np.any(br), "nonzero router bias unsupported"

    x16 = x.astype(np.float16)
    # x32t[p, b, c, n] = x[b*512 + n, c*128 + p]  (fp32: exact router)
    x32t = np.ascontiguousarray(
        x.reshape(NB, BT, DC, P).transpose(3, 0, 2, 1)
    )

    p = np.arange(P)
    c16 = np.zeros((P, C16_W), np.float16)
    c16[:, C16_IDENT:C16_IDENT + P] = np.eye(P, dtype=np.float16)
    c16[:, C16_TRI:C16_TRI + P] = (p[:, None] < p[None, :]).astype(np.float16)
    c16[:, C16_ONES] = 1.0

    cf32 = np.zeros((P, CF_W), np.float32)
    cf32[:, CF_IOTAC:CF_IOTAC + NT] = (
        np.arange(NT)[None, :] * P + p[:, None]
    ).astype(np.float32)
    cf32[:, CF_IOTAW:CF_IOTAW + RT * 8] = (
        np.arange(RT * 8)[None, :] * 16 + (p % 16)[:, None]
    ).astype(np.float32)
    cf32[:, CF_WR:CF_WR + DC * E] = (
        Wr.reshape(DC, P, E).transpose(1, 0, 2).reshape(P, DC * E)
    )

    sidx0 = np.zeros((N_TOKENS, REC), np.float32)
    selg = np.zeros((P, 8, P), np.float32)
    g_idx = np.arange(8)
    for pp in range(P):
        selg[g_idx * 16 + (pp % 16), g_idx, pp] = 1.0

    shared = dict(
        x16=np.ascontiguousarray(x16), x32t=x32t, c16=c16, cf32=cf32,
        sidx=sidx0, selg=selg,
    )

    in_maps = []
    for e in range(E):
        m = dict(shared)
        w1e = W1[e].astype(np.float16)
        # w1t[p, jb, c, j] = W1[c*128 + p, jb*512 + j]
        m["w1t"] = np.ascontiguousarray(
            w1e.reshape(DC, P, NJB, JB).transpose(1, 2, 0, 3)
        )
        w2e = W2[e].astype(np.float16)
        # w2t[p, wb, g, k] = W2[(wb*8 + g)*128 + p, k]
        m["w2t"] = np.ascontiguousarray(
            w2e.reshape(NWB, JG // NWB, P, D_OUT).transpose(2, 0, 1, 3)
        )
        cr32 = np.zeros((1, CR_W), np.float32)
        cr32[0, CR_ONES:CR_ONES + P] = 1.0
        cr32[0, CR_ONEHOT + e] = 1.0
        m["cr32"] = cr32
        in_maps.append(m)
    return in_maps


def _combine(results):
    out = np.zeros((N_TOKENS, D_OUT), np.float32)
    cnts = results[0]["cnts"][0]
    total = 0
    for e in range(E):
        n = int(round(float(cnts[e])))
        assert 0 <= n <= CAP, f"expert {e} count {n} exceeds capacity {CAP}"
        idx = results[e]["ids5"].T.reshape(CAP)[:n].astype(np.int64)
        arr = results[e]["outT16"].reshape(P, KC, CAP)
        rows = np.transpose(arr, (2, 1, 0)).reshape(CAP, KC * P).astype(np.float32)
        out[idx] = rows[:n]
        total += n
    assert total == N_TOKENS, f"token counts sum to {total}, expected {N_TOKENS}"
    return out


def kernel(**inputs) -> np.ndarray:
    nc = _get_nc()
    in_maps = _make_in_maps(**inputs)
    res = run_bass_kernel_spmd(nc, in_maps, core_ids=list(range(E)))
    return _combine(res.results)


def kernel_traced(**inputs):
    """Like kernel() but with NTFF profiling; returns (out, BassKernelResults)."""
    nc = _get_nc()
    in_maps = _make_in_maps(**inputs)
    res = run_bass_kernel_spmd(
        nc, in_maps, core_ids=list(range(E)), trace=True
    )
    return _combine(res.results), res



# revision 6
# speedup vs baseline: 1.0724x; 1.0724x over previous
"""Trainium2 Bass kernel for nn_MoE_AllToAll_Layer (top-1 MoE, 8 experts).

Expert parallel across 8 NeuronCores: core e holds expert e's FFN weights.
Core e computes the router for ITS OWN 512-token slice only (2MB fp32 x
stream instead of a replicated 16MB one), broadcasts the per-token
(argmax-code, score) pairs to all cores with one small AllGather, then each
core rebuilds the full routing state and runs the unchanged counting-sort /
scatter / gather / FFN pipeline:
 - stable counting sort of all 4096 tokens by expert, scatter of
   (token_id, score) records into a sorted-position-indexed DRAM array with
   ONE dma_scatter_add, gather of the core's own expert rows with
   dma_gather, expert FFN on the compacted tokens, compact scaled output
   rows + token ids written out; the host places rows back by token id
   (pure data movement).

Validated-on-HW design notes (carried over from the replicated-router
version, which this supersedes):
 - The router is exact fp32 (an fp16 router flips ~1 argmax on these inputs,
   and one flip shifts the reference's positional score permutation, which
   corrupts hundreds of rows). x is streamed fp32 but used as the PE's
   stationary operand with the tiny Wr moving, so fp32's 4 cycles/row apply
   only to 8-wide outputs. Only the 512-token slice is routed per core; the
   broadcast payload (renc in {1..8}, score) is exact in fp16 for renc and
   adds one fp16 rounding to score (score is already applied in fp16 at the
   diag step, so the extra rounding is noise vs the 2e-2 gate).
 - AllGather payload layout: pk[p, 0:4]=renc, pk[p, 4:8]=score is
   PE-transposed to [8, 128] rows, AllGather concatenates cores to
   [64, 128], and two static selector matmuls ([64, 32] one-hots) transpose
   renc/score back to [P, NT] in one shot.
 - FFN weights/activations/outputs are fp16 (1 cycle/row, half the DMA).
 - dma_gather/dma_scatter_add index layout: [128, n/16] int16, the
   [16, n/16] wrap (slot i at [i%16, i//16]) replicated 8x down partitions
   (one copy per Q7 core); built on the PE with 8 selection matmuls.
   Gather writes slot i to out[i%128, i//128].
 - The reference's positional score scale is folded into the PE transpose
   of the gathered x rows via a plain matmul with diag(score) as the rhs.
 - FFN: layer-1 for all hidden blocks (streaming W1), then layer-2
   accumulating all 32 hidden chunks in PSUM (W2 streamed during L1).
   Both layers keep the stationary weight tile across the two token chunks
   to halve Ldweights issue cost.
 - All DMAs contend on one DMA_ENGINES resource: the W1/W2 streams are
   gated behind the xg gather (first W1 tile corner-copied from xg) so the
   scheduler cannot hoist 16MB of weight traffic ahead of the small
   latency-critical scatter/gather chain.
"""

import numpy as np
import sys

sys.path.insert(0, "/opt/trn_rl_repo")

import concourse.bass as bass  # noqa: E402
import concourse.tile as tile  # noqa: E402
from concourse import bacc, mybir  # noqa: E402
from concourse.bass_utils import run_bass_kernel_spmd  # noqa: E402

P = 128
N_TOKENS = 4096
D_IN = 1024
D_HID = 4096
D_OUT = 1024
E = 8
NT = N_TOKENS // P          # 32 token tiles
DC = D_IN // P              # 8 d-chunks
KC = D_OUT // P             # 8 k-chunks
JG = D_HID // P             # 32 hidden chunks
CAP = 640                   # per-expert token capacity (counts are platform-
                            # dependent: 536 max on cpu-generated inputs, 583
                            # on device-generated ones — keep slack)
RT = CAP // P               # 5 row tiles
BT = N_TOKENS // E          # 512 tokens per core slice
TPB = BT // P               # 4 token tiles per slice
NJB = 8                     # W1 streaming blocks (512 hidden each)
JB = D_HID // NJB           # 512
JCB = JB // P               # 4 hidden chunks per W1 block
NWB = 4                     # W2 streaming blocks (8 hidden chunks each)
CHUNKS = ((0, 512), (512, CAP - 512))   # token chunks for FFN matmuls
REC = 64                    # f32 row stride of sidx records (256B min)

dt = mybir.dt
Alu = mybir.AluOpType
Act = mybir.ActivationFunctionType
Ax = mybir.AxisListType

f32 = dt.float32
f16 = dt.float16
i16 = dt.int16

# const blob column offsets
C16_IDENT = 0            # [0:128)   eye(128) fp16
C16_TRI = 128            # [128:256) tri[q,p] = q < p
C16_ONES = 256           # [256:257) 1.0
C16_SELR = 257           # [257:289) renc-row selector [64, 32]
C16_SELS = 289           # [289:321) score-row selector [64, 32]
C16_W = 321
CF_IOTAC = 0             # [0:32)  iotac[p, t] = t*128 + p
CF_IOTAW = 32            # [32:72) iotaw[p, m] = 16*m + p%16 (wrapped iota)
CF_WR = 72               # [72:136) wr32[p, c*8+e] = Wr[c*128+p, e] (fp32!)
CF_W = 136
CR_ONES = 0              # [0:128) ones
CR_ONEHOT = 128          # [128:136) onehot(core expert)
CR_W = 136


def build_nc():
    nc = bacc.Bacc(
        "TRN2",
        target_bir_lowering=False,
        debug=False,
        enable_asserts=False,
        num_devices=E,
    )

    # per-core router slice: x32s[p, c, n] = x[e*512 + n, c*128 + p]
    x32s = nc.dram_tensor("x32s", [P, DC, BT], f32, kind="ExternalInput").ap()
    x16 = nc.dram_tensor("x16", [N_TOKENS, D_IN], f16, kind="ExternalInput").ap()
    w1t = nc.dram_tensor("w1t", [P, NJB, DC, JB], f16, kind="ExternalInput").ap()
    w2t = nc.dram_tensor("w2t", [P, NWB, JG // NWB, D_OUT], f16, kind="ExternalInput").ap()
    c16 = nc.dram_tensor("c16", [P, C16_W], f16, kind="ExternalInput").ap()
    cf32 = nc.dram_tensor("cf32", [P, CF_W], f32, kind="ExternalInput").ap()
    cr32 = nc.dram_tensor("cr32", [1, CR_W], f32, kind="ExternalInput").ap()
    # pre-zeroed scatter destination (host ships zeros)
    sidx = nc.dram_tensor("sidx", [N_TOKENS, REC], f32, kind="ExternalInput").ap()
    # wrap/replicate selector: selg[p, g, q] = 1 iff p == 16*g + (q % 16)
    selg = nc.dram_tensor("selg", [P, 8, P], f32, kind="ExternalInput").ap()

    outT16 = nc.dram_tensor("outT16", [P, KC, CAP], f16, kind="ExternalOutput").ap()
    ids5 = nc.dram_tensor("ids5", [P, RT], f32, kind="ExternalOutput").ap()
    cnts = nc.dram_tensor("cnts", [1, E], f32, kind="ExternalOutput").ap()

    with tile.TileContext(nc) as tc:
        emit(nc, tc, locals())
    nc.compile()
    return nc


def emit(nc, tc, io):
    x32s, x16, w1t, w2t = io["x32s"], io["x16"], io["w1t"], io["w2t"]
    c16, cf32, cr32 = io["c16"], io["cf32"], io["cr32"]
    outT16, ids5, cnts = io["outT16"], io["ids5"], io["cnts"]
    sidx = io["sidx"]

    with tc.tile_pool(name="consts", bufs=1) as cpool:
        cf_sb = cpool.tile([P, CF_W], f32, tag="cf32")
        nc.sync.dma_start(cf_sb[:], cf32)
        c16_sb = cpool.tile([P, C16_W], f16, tag="c16")
        cr_sb = cpool.tile([1, CR_W], f32, tag="cr32")
        selg_sb = cpool.tile([P, 8, P], f32, tag="selg")

        ident16 = c16_sb[:, C16_IDENT:C16_IDENT + P]
        tri16 = c16_sb[:, C16_TRI:C16_TRI + P]
        ones1_16 = c16_sb[:, C16_ONES:C16_ONES + 1]
        sel_r = c16_sb[0:64, C16_SELR:C16_SELR + NT]
        sel_s = c16_sb[0:64, C16_SELS:C16_SELS + NT]
        iotac = cf_sb[:, CF_IOTAC:CF_IOTAC + NT]
        iotaw = cf_sb[:, CF_IOTAW:CF_IOTAW + RT * 8]
        wr32 = cf_sb[:, CF_WR:CF_WR + DC * E]
        onesr = cr_sb[:, CR_ONES:CR_ONES + P]
        onehot = cr_sb[:, CR_ONEHOT:CR_ONEHOT + E]

        with (
            tc.tile_pool(name="persist", bufs=1) as pp,
            tc.tile_pool(name="dramb", bufs=1, space="DRAM") as dram,
        ):
            renc = pp.tile([P, NT], f32, tag="renc")
            m_all = pp.tile([P, NT, E], f32, tag="mall")
            m16 = pp.tile([P, NT * E], f16, tag="m16")
            own_bc = pp.tile([P, 1], f32, tag="ownbc")
            xT_all = pp.tile([P, DC, CAP], f16, tag="xTall")
            hT_all = pp.tile([P, JG, CAP], f16, tag="hTall")
            o16 = pp.tile([P, KC, CAP], f16, tag="o16")
            s2ro = pp.tile([P, RT, REC], f32, tag="s2ro")
            w2_all = pp.tile([P, JG, D_OUT], f16, tag="w2all")
            sc_big = pp.tile([P, NT, 2], f32, tag="scbig")
            bounce_in = dram.tile([E, P], f16, tag="bin")
            bounce_out = dram.tile([E * E, P], f16, tag="bout")

            # scatter records: col0 = token id (known now), col1 = score
            nc.vector.tensor_copy(out=sc_big[:, :, 0], in_=iotac)

            # ---------------- router: own 512-token slice, exact fp32 --------
            # logits[tok, e] with x chunks as the (free) PE weights and the
            # tiny Wr as the moving operand. c-outer with 4 concurrent PSUM
            # accumulators so the PE trails the 4-chunk x DMA stream.
            with (
                tc.tile_pool(name="rwork", bufs=1) as rp,
                tc.tile_pool(name="tpsum", bufs=1, space="PSUM") as tps,
            ):
                xtt = rp.tile([P, DC, BT], f32, tag="xtt")
                for cc in range(4):
                    nc.sync.dma_start(
                        xtt[:, 2 * cc:2 * cc + 2, :], x32s[:, 2 * cc:2 * cc + 2, :]
                    )
                # remaining consts ride the sync queue behind the x slice
                nc.sync.dma_start(c16_sb[:], c16)
                nc.sync.dma_start(cr_sb[:], cr32)
                nc.sync.dma_start(selg_sb[:], io["selg"])

                lg_ps = []
                for i in range(TPB):
                    lg_ps_i = tps.tile([P, 8], f32, tag=f"lgps{i}", name=f"lgps{i}")
                    lg_ps.append(lg_ps_i)
                for c in range(DC):
                    for i in range(TPB):
                        nc.tensor.matmul(
                            lg_ps[i][:],
                            lhsT=xtt[:, c, i * P:(i + 1) * P],
                            rhs=wr32[:].rearrange("p (c e) -> p c e", c=DC)[:, c, :],
                            start=(c == 0), stop=(c == DC - 1),
                        )
                lg_sl = rp.tile([P, TPB, E], f32, tag="lgsl")
                for i in range(TPB):
                    nc.vector.tensor_copy(out=lg_sl[:, i, :], in_=lg_ps[i][:])

                mx_sl = rp.tile([P, TPB], f32, tag="mxsl")
                nc.vector.tensor_reduce(
                    out=mx_sl[:], in_=lg_sl[:], axis=Ax.X, op=Alu.max
                )
                # renc = max_e (lg == mx) * (E - e)  (first-max tiebreak)
                renc_sl = rp.tile([P, TPB], f32, tag="rencsl")
                for e in range(E):
                    eq = rp.tile([P, TPB], f32, tag="eq", bufs=2)
                    nc.vector.tensor_tensor(
                        out=eq[:], in0=lg_sl[:, :, e], in1=mx_sl[:],
                        op=Alu.is_equal,
                    )
                    if e == 0:
                        nc.vector.tensor_scalar_mul(renc_sl[:], eq[:], float(E))
                    else:
                        eqr = rp.tile([P, TPB], f32, tag="eqr", bufs=2)
                        nc.vector.tensor_scalar_mul(eqr[:], eq[:], float(E - e))
                        nc.vector.tensor_tensor(
                            out=renc_sl[:], in0=renc_sl[:], in1=eqr[:],
                            op=Alu.max,
                        )
                el = rp.tile([P, TPB, E], f32, tag="el")
                nc.scalar.activation(el[:], lg_sl[:], Act.Exp)
                ssum = rp.tile([P, TPB], f32, tag="ssum")
                nc.vector.tensor_reduce(
                    out=ssum[:], in_=el[:], axis=Ax.X, op=Alu.add
                )
                emx = rp.tile([P, TPB], f32, tag="emx")
                nc.scalar.activation(emx[:], mx_sl[:], Act.Exp)
                rsum = rp.tile([P, TPB], f32, tag="rsum")
                nc.vector.reciprocal(rsum[:], ssum[:])
                score_sl = rp.tile([P, TPB], f32, tag="scoresl")
                nc.vector.tensor_tensor(
                    out=score_sl[:], in0=emx[:], in1=rsum[:], op=Alu.mult
                )

                # pack (renc | score) and PE-transpose to [8, 128] rows
                pk = rp.tile([P, 2 * TPB], f16, tag="pk")
                nc.vector.tensor_copy(out=pk[:, 0:TPB], in_=renc_sl[:])
                nc.vector.tensor_copy(out=pk[:, TPB:2 * TPB], in_=score_sl[:])
                with tc.tile_pool(name="pkps", bufs=1, space="PSUM") as pkps:
                    ps8 = pkps.tile([2 * TPB, P], f32, tag="ps8")
                    nc.tensor.matmul(
                        ps8[:], lhsT=pk[:], rhs=ident16, start=True, stop=True
                    )
                    s8 = rp.tile([2 * TPB, P], f16, tag="s8")
                    nc.vector.tensor_copy(out=s8[:], in_=ps8[:])
                nc.sync.dma_start(bounce_in[:], s8[:])

                # broadcast routing decisions: [8, 128] -> [64, 128]
                nc.gpsimd.collective_compute(
                    "AllGather",
                    Alu.bypass,
                    replica_groups=[list(range(E))],
                    ins=[bounce_in.opt()],
                    outs=[bounce_out.opt()],
                )

                # reload + selector-transpose back to [P, NT]
                ld = rp.tile([E * E, P], f16, tag="ld")
                nc.sync.dma_start(ld[:], bounce_out[:])
                with tc.tile_pool(name="ldps", bufs=2, space="PSUM") as ldps:
                    ps_r = ldps.tile([P, NT], f32, tag="psr")
                    nc.tensor.matmul(
                        ps_r[:], lhsT=ld[:], rhs=sel_r, start=True, stop=True
                    )
                    nc.vector.tensor_copy(out=renc[:], in_=ps_r[:])
                    ps_s = ldps.tile([P, NT], f32, tag="pss")
                    nc.tensor.matmul(
                        ps_s[:], lhsT=ld[:], rhs=sel_s, start=True, stop=True
                    )
                    nc.vector.tensor_copy(out=sc_big[:, :, 1], in_=ps_s[:])

                # sort masks for all 32 t-tiles
                for e in range(E):
                    nc.vector.tensor_scalar(
                        out=m_all[:, :, e], in0=renc[:],
                        scalar1=float(E - e), scalar2=None, op0=Alu.is_equal,
                    )
                nc.vector.tensor_copy(out=m16[:], in_=m_all[:])

            # ---------------- stable counting sort ----------------
            with (
                tc.tile_pool(name="swork", bufs=1) as sw,
                tc.tile_pool(name="spsum", bufs=1, space="PSUM") as sps,
            ):
                prefix_ps = sps.tile([P, NT, E], f32, tag="prefix")
                nc.tensor.matmul(
                    prefix_ps[:].rearrange("p t e -> p (t e)"),
                    lhsT=tri16, rhs=m16[:], start=True, stop=True,
                )
                colsum_ps = sps.tile([1, NT * E], f32, tag="colsum")
                nc.tensor.matmul(
                    colsum_ps[:], lhsT=ones1_16, rhs=m16[:],
                    start=True, stop=True,
                )
                cs = sw.tile([1, NT, E], f32, tag="cs")
                nc.vector.tensor_copy(
                    out=cs[:].rearrange("p t e -> p (t e)"), in_=colsum_ps[:]
                )
                # counts independent of the scan: reduce over t
                csT = sw.tile([1, E, NT], f32, tag="csT")
                nc.vector.tensor_copy(
                    out=csT[:], in_=cs[:].rearrange("p t e -> p e t")
                )
                cnt_row = sw.tile([1, E], f32, tag="cnt")
                nc.vector.tensor_reduce(
                    out=cnt_row[:], in_=csT[:], axis=Ax.X, op=Alu.add
                )
                nc.scalar.dma_start(cnts, cnt_row[:])

                # exclusive prefix over experts -> global offsets
                ocur = sw.tile([1, E], f32, tag="off0")
                nc.vector.memset(ocur[:], 0.0)
                nc.vector.tensor_copy(out=ocur[:, 1:E], in_=cnt_row[:, 0:E - 1])
                for i, s in enumerate([1, 2, 4]):
                    onxt = sw.tile([1, E], f32, tag=f"off{i + 1}")
                    nc.vector.tensor_tensor(
                        out=onxt[:, s:E], in0=ocur[:, s:E],
                        in1=ocur[:, 0:E - s], op=Alu.add,
                    )
                    nc.vector.tensor_copy(out=onxt[:, 0:s], in_=ocur[:, 0:s])
                    ocur = onxt
                off_row = ocur  # [1, E]

                oh = sw.tile([1, E], f32, tag="oh")
                nc.vector.tensor_tensor(
                    out=oh[:], in0=off_row[:], in1=onehot, op=Alu.mult
                )
                own1 = sw.tile([1, 1], f32, tag="own1")
                nc.vector.tensor_reduce(
                    out=own1[:], in_=oh[:], axis=Ax.X, op=Alu.add
                )
                ownb_ps = sps.tile([P, 1], f32, tag="ownb")
                nc.tensor.matmul(
                    ownb_ps[:], lhsT=onesr, rhs=own1[:], start=True, stop=True
                )
                nc.vector.tensor_copy(out=own_bc[:], in_=ownb_ps[:])

                # within-expert exclusive prefix over t, SEEDED with the
                # global expert offsets so carry2 = off_e + sum_{t'<t} cs
                cur = sw.tile([1, NT, E], f32, tag="hs0")
                nc.vector.tensor_copy(out=cur[:, 0:1, :], in_=off_row[:])
                nc.vector.tensor_copy(
                    out=cur[:, 1:NT, :], in_=cs[:, 0:NT - 1, :]
                )
                for i, s in enumerate([1, 2, 4, 8, 16]):
                    nxt = sw.tile([1, NT, E], f32, tag=f"hs{i + 1}")
                    nc.vector.tensor_tensor(
                        out=nxt[:, s:NT, :], in0=cur[:, s:NT, :],
                        in1=cur[:, 0:NT - s, :], op=Alu.add,
                    )
                    nc.vector.tensor_copy(out=nxt[:, 0:s, :], in_=cur[:, 0:s, :])
                    cur = nxt
                carry2 = cur  # [1, t, e] = off_e + exclusive within-e prefix

                carb_ps = sps.tile([P, NT, E], f32, tag="carb")
                nc.tensor.matmul(
                    carb_ps[:].rearrange("p t e -> p (t e)"),
                    lhsT=onesr, rhs=carry2[:].rearrange("p t e -> p (t e)"),
                    start=True, stop=True,
                )

                # dest[p, t] = sum_e m_e * (prefix_e + carry2_e)
                # (hardware allows only one PSUM input per vector op)
                carb_sb = sw.tile([P, NT, E], f32, tag="carbsb")
                nc.vector.tensor_copy(out=carb_sb[:], in_=carb_ps[:])
                s1 = sw.tile([P, NT, E], f32, tag="s1")
                nc.vector.tensor_tensor(
                    out=s1[:], in0=prefix_ps[:], in1=carb_sb[:], op=Alu.add
                )
                s2 = sw.tile([P, NT, E], f32, tag="s2")
                nc.vector.tensor_tensor(
                    out=s2[:], in0=s1[:], in1=m_all[:], op=Alu.mult
                )
                dest = sw.tile([P, NT], f32, tag="dest")
                nc.vector.tensor_reduce(
                    out=dest[:], in_=s2[:], axis=Ax.X, op=Alu.add
                )

                # wrap + replicate scatter indices on the PE: slot i = t*128+p
                # lives at [i%16 (+16c), t*8 + p//16]; the selection matmul
                # moves dest[16g + q%16, t] to partition q, column group g,
                # replicated for all 8 Q7 cores at once.
                dest16w = sw.tile([P, NT, 8], i16, tag="dest16w")
                with tc.tile_pool(name="wps", bufs=2, space="PSUM") as wps:
                    for g in range(8):
                        wp_ps = wps.tile([P, NT], f32, tag="wpps")
                        nc.tensor.matmul(
                            wp_ps[:], lhsT=selg_sb[:, g, :], rhs=dest[:],
                            start=True, stop=True,
                        )
                        nc.vector.tensor_copy(
                            out=dest16w[:, :, g], in_=wp_ps[:]
                        )
                nc.gpsimd.dma_scatter_add(
                    sidx[:, 0:2], sc_big[:],
                    dest16w[:].rearrange("p t g -> p (t g)"),
                    N_TOKENS, N_TOKENS, 2, elem_step=REC,
                )

            # ---------------- gather own rows + scaled transpose -------------
            with tc.tile_pool(name="gwork", bufs=1) as gp:
                # own sorted positions, wrapped layout, computed in place
                posw = gp.tile([P, RT * 8], f32, tag="posw")
                nc.vector.tensor_scalar(
                    out=posw[:], in0=iotaw,
                    scalar1=own_bc[:, 0:1], scalar2=float(N_TOKENS - 1),
                    op0=Alu.add, op1=Alu.min,
                )
                pos16w = gp.tile([P, RT * 8], i16, tag="pos16w")
                nc.vector.tensor_copy(out=pos16w[:], in_=posw[:])
                sgo = gp.tile([P, RT, REC], f32, tag="sgo")
                nc.gpsimd.dma_gather(
                    sgo[:], sidx, pos16w[:], CAP, CAP, REC,
                )
                nc.scalar.dma_start(ids5, sgo[:, :, 0])
                ids16w = gp.tile([P, RT, 8], i16, tag="ids16w")
                with tc.tile_pool(name="iwps", bufs=2, space="PSUM") as iwps:
                    for g in range(8):
                        iw_ps = iwps.tile([P, RT], f32, tag="iwps")
                        nc.tensor.matmul(
                            iw_ps[:], lhsT=selg_sb[:, g, :], rhs=sgo[:, :, 0],
                            start=True, stop=True,
                        )
                        nc.vector.tensor_copy(
                            out=ids16w[:, :, g], in_=iw_ps[:]
                        )
                # scale lookup: sorted_scores[token_id]
                nc.gpsimd.dma_gather(
                    s2ro[:], sidx,
                    ids16w[:].rearrange("p r g -> p (r g)"), CAP, CAP, REC,
                )
                xg = gp.tile([P, RT, D_IN], f16, tag="xg")
                idsw_flat = ids16w[:].rearrange("p r g -> p (r g)")
                nc.gpsimd.dma_gather(
                    xg[:, 0:3, :], x16, idsw_flat[:, 0:24], 384, 384, D_IN,
                )
                nc.gpsimd.dma_gather(
                    xg[:, 3:RT, :], x16, idsw_flat[:, 24:40], 256, 256, D_IN,
                )
                # gate the W1/W2 streams behind xg: without this the
                # scheduler hoists 16MB of weight traffic ahead of the
                # scatter/gather chain, stalling the prefix (all DMAs
                # contend on one DMA_ENGINES resource)
                nc.vector.tensor_copy(
                    out=w2_all[0:1, 0:1, 0:1], in_=xg[0:1, 0:1, 0:1]
                )
                # transpose gathered rows, folding the positional score scale
                # in by multiplying with diag(score) on the PE
                # (exact because b1 = b2 = 0 and scores > 0)
                with tc.tile_pool(name="tpx", bufs=8, space="PSUM") as tpx:
                    for rt in range(RT):
                        diag = gp.tile([P, P], f16, tag="diag", bufs=2)
                        nc.vector.tensor_scalar(
                            out=diag[:], in0=ident16,
                            scalar1=s2ro[:, rt, 1:2], scalar2=None, op0=Alu.mult,
                        )
                        for c in range(DC):
                            tp = tpx.tile([P, P], f32, tag="tp")
                            nc.tensor.matmul(
                                tp[:],
                                lhsT=xg[:, rt, c * P:(c + 1) * P],
                                rhs=diag[:],
                                start=True, stop=True,
                            )
                            if c % 2 == 0:
                                nc.vector.tensor_copy(
                                    out=xT_all[:, c, rt * P:(rt + 1) * P],
                                    in_=tp[:],
                                )
                            else:
                                nc.scalar.activation(
                                    xT_all[:, c, rt * P:(rt + 1) * P],
                                    tp[:], Act.Copy,
                                )

            # ---------------- FFN layer 1 (stream W1, then W2) ---------------
            # c-outer so both token chunks reuse the stationary W1 tile
            with (
                tc.tile_pool(name="w1pool", bufs=3) as wp,
                tc.tile_pool(name="l1ps", bufs=3, space="PSUM") as l1ps,
                tc.tile_pool(name="l1tail", bufs=3, space="PSUM") as l1tail,
            ):
                for jb in range(NJB):
                    w1b = wp.tile([P, DC, JB], f16, tag="w1b")
                    if jb == 0:
                        # second hop of the weight-stream gate (w2_all corner
                        # was corner-copied from xg inside the gather scope)
                        nc.vector.tensor_copy(
                            out=w1b[0:1, 0:1, 0:1], in_=w2_all[0:1, 0:1, 0:1]
                        )
                    nc.sync.dma_start(w1b[:], w1t[:, jb])
                    if 3 <= jb <= 6:
                        wb = jb - 3
                        nc.sync.dma_start(
                            w2_all[:, wb * 8:(wb + 1) * 8, :], w2t[:, wb]
                        )
                    for jc in range(JCB):
                        jg = jb * JCB + jc
                        ps_a = l1ps.tile([P, 512], f32, tag="l1pa")
                        ps_b = l1tail.tile([P, CAP - 512], f32, tag="l1pb")
                        for c in range(DC):
                            lhsT = w1b[:, c, jc * P:(jc + 1) * P]
                            nc.tensor.matmul(
                                ps_a[:], lhsT=lhsT,
                                rhs=xT_all[:, c, 0:512],
                                start=(c == 0), stop=(c == DC - 1),
                            )
                            nc.tensor.matmul(
                                ps_b[:], lhsT=lhsT,
                                rhs=xT_all[:, c, 512:CAP],
                                start=(c == 0), stop=(c == DC - 1),
                            )
                        nc.scalar.activation(
                            hT_all[:, jg, 0:512], ps_a[:], Act.Relu
                        )
                        nc.scalar.activation(
                            hT_all[:, jg, 512:CAP], ps_b[:], Act.Relu
                        )

            # ---------------- FFN layer 2 (full PSUM accumulation) -----------
            # g-outer with both chunks inner: one Ldweights per (g, kc)
            with (
                tc.tile_pool(name="l2ps", bufs=2, space="PSUM") as l2ps,
                tc.tile_pool(name="l2tail", bufs=2, space="PSUM") as l2tail,
            ):
                for kc in range(KC):
                    ps_a = l2ps.tile([P, 512], f32, tag="l2pa")
                    ps_b = l2tail.tile([P, CAP - 512], f32, tag="l2pb")
                    for g in range(JG):
                        lhsT = w2_all[:, g, kc * P:(kc + 1) * P]
                        nc.tensor.matmul(
                            ps_a[:], lhsT=lhsT, rhs=hT_all[:, g, 0:512],
                            start=(g == 0), stop=(g == JG - 1),
                        )
                        nc.tensor.matmul(
                            ps_b[:], lhsT=lhsT, rhs=hT_all[:, g, 512:CAP],
                            start=(g == 0), stop=(g == JG - 1),
                        )
                    nc.vector.tensor_copy(out=o16[:, kc, 0:512], in_=ps_a[:])
                    nc.vector.tensor_copy(out=o16[:, kc, 512:CAP], in_=ps_b[:])
                    nc.sync.dma_start(outT16[:, kc, :], o16[:, kc, :])


_NC_CACHE = None


def _get_nc():
    global _NC_CACHE
    if _NC_CACHE is None:
        _NC_CACHE = build_nc()
    return _NC_CACHE


def _make_in_maps(x, Wr, br, W1, b1, W2, b2):
    x = np.asarray(x, np.float32)
    Wr = np.asarray(Wr, np.float32)
    br = np.asarray(br, np.float32)
    W1 = np.asarray(W1, np.float32)
    W2 = np.asarray(W2, np.float32)
    b1 = np.asarray(b1, np.float32)
    b2 = np.asarray(b2, np.float32)
    # the kernel folds the positional score scale onto x and drops the FFN
    # bias adds, which is exact only for zero biases (the spec generates
    # zeros)
    assert not np.any(b1) and not np.any(b2), "nonzero FFN biases unsupported"
    assert not np.any(br), "nonzero router bias unsupported"

    x16 = x.astype(np.float16)
    # x32s[e][p, c, n] = x[e*512 + n, c*128 + p]  (fp32: exact router)
    x_resh = x.reshape(E, BT, DC, P)

    p = np.arange(P)
    c16 = np.zeros((P, C16_W), np.float16)
    c16[:, C16_IDENT:C16_IDENT + P] = np.eye(P, dtype=np.float16)
    c16[:, C16_TRI:C16_TRI + P] = (p[:, None] < p[None, :]).astype(np.float16)
    c16[:, C16_ONES] = 1.0
    # AllGather row selectors: row 8s+j is renc t-tile 4s+j, row 8s+4+j is
    # score t-tile 4s+j
    s_idx = np.arange(E)
    j_idx = np.arange(TPB)
    for s in s_idx:
        for j in j_idx:
            c16[8 * s + j, C16_SELR + 4 * s + j] = 1.0
            c16[8 * s + 4 + j, C16_SELS + 4 * s + j] = 1.0

    cf32 = np.zeros((P, CF_W), np.float32)
    cf32[:, CF_IOTAC:CF_IOTAC + NT] = (
        np.arange(NT)[None, :] * P + p[:, None]
    ).astype(np.float32)
    cf32[:, CF_IOTAW:CF_IOTAW + RT * 8] = (
        np.arange(RT * 8)[None, :] * 16 + (p % 16)[:, None]
    ).astype(np.float32)
    cf32[:, CF_WR:CF_WR + DC * E] = (
        Wr.reshape(DC, P, E).transpose(1, 0, 2).reshape(P, DC * E)
    )

    sidx0 = np.zeros((N_TOKENS, REC), np.float32)
    selg = np.zeros((P, 8, P), np.float32)
    g_idx = np.arange(8)
    for pp in range(P):
        selg[g_idx * 16 + (pp % 16), g_idx, pp] = 1.0

    shared = dict(
        x16=np.ascontiguousarray(x16), c16=c16, cf32=cf32,
        sidx=sidx0, selg=selg,
    )

    in_maps = []
    for e in range(E):
        m = dict(shared)
        m["x32s"] = np.ascontiguousarray(x_resh[e].transpose(2, 1, 0))
        w1e = W1[e].astype(np.float16)
        # w1t[p, jb, c, j] = W1[c*128 + p, jb*512 + j]
        m["w1t"] = np.ascontiguousarray(
            w1e.reshape(DC, P, NJB, JB).transpose(1, 2, 0, 3)
        )
        w2e = W2[e].astype(np.float16)
        # w2t[p, wb, g, k] = W2[(wb*8 + g)*128 + p, k]
        m["w2t"] = np.ascontiguousarray(
            w2e.reshape(NWB, JG // NWB, P, D_OUT).transpose(2, 0, 1, 3)
        )
        cr32 = np.zeros((1, CR_W), np.float32)
        cr32[0, CR_ONES:CR_ONES + P] = 1.0
        cr32[0, CR_ONEHOT + e] = 1.0
        m["cr32"] = cr32
        in_maps.append(m)
    return in_maps


def _combine(results):
    out = np.zeros((N_TOKENS, D_OUT), np.float32)
    cnts = results[0]["cnts"][0]
    total = 0
    for e in range(E):
        n = int(round(float(cnts[e])))
        assert 0 <= n <= CAP, f"expert {e} count {n} exceeds capacity {CAP}"
        idx = results[e]["ids5"].T.reshape(CAP)[:n].astype(np.int64)
        arr = results[e]["outT16"].reshape(P, KC, CAP)
        rows = np.transpose(arr, (2, 1, 0)).reshape(CAP, KC * P).astype(np.float32)
        out[idx] = rows[:n]
        total += n
    assert total == N_TOKENS, f"token counts sum to {total}, expected {N_TOKENS}"
    return out


def kernel(**inputs) -> np.ndarray:
    nc = _get_nc()
    in_maps = _make_in_maps(**inputs)
    res = run_bass_kernel_spmd(nc, in_maps, core_ids=list(range(E)))
    return _combine(res.results)


def kernel_traced(**inputs):
    """Like kernel() but with NTFF profiling; returns (out, BassKernelResults)."""
    nc = _get_nc()
    in_maps = _make_in_maps(**inputs)
    res = run_bass_kernel_spmd(
        nc, in_maps, core_ids=list(range(E)), trace=True
    )
    return _combine(res.results), res
